# revision 30
# baseline (speedup 1.0000x reference)
"""Trainium2 Bass/Tile kernel for a dense transformer block (B=2, T=2048, D=1024, H=16).

Sharding across 8 NeuronCores, designed to avoid large collectives:
  - LayerNorm1 statistics are computed (replicated) from the transposed input
    via PE-matmul column reductions; the LN *apply* is algebraically folded
    into each core's Q/K/V projections (X = A*X_raw + C*wsum + wb).
  - Attention is head-sharded: each core owns 2 of the 16 heads for both
    batches. The reference's query-axis softmax is reformulated as
      attn^T = (V/c)^T @ exp(S^T),  c[k] = row-sum of exp(S^T)
    so the normalizer is a cheap free-axis reduction folded into V.
  - One 2MB/core AllToAll reshards attention output from head-split to
    token-split (512 tokens per core).
  - Projection, LN2 and the FFN run token-sharded with replicated, streamed
    W1/W2 weights.
Everything is computed in feature-major ("transposed") layout so every matmul
has its natural lhsT operand; the host supplies x^T and re-transposes the
512-token output slices at the end.
"""

import numpy as np
import ml_dtypes

import concourse.bass as bass
import concourse.bacc as bacc
import concourse.mybir as mybir
import concourse.tile as tile
from concourse.bass_utils import run_bass_kernel_spmd

F32 = mybir.dt.float32
BF16 = mybir.dt.bfloat16
AF = mybir.ActivationFunctionType
ALU = mybir.AluOpType

B, T, D, H = 2, 2048, 1024, 16
HS = D // H          # 64
DFF = 4 * D          # 4096
EPS = 1e-5
NC_ = 8              # cores
BT = B * T           # 4096 flat tokens
TOK = BT // NC_      # 512 tokens per core
NBLK = BT // 512     # 8 token blocks
NDC = D // 128       # 8 d-chunks
NH_LOC = H // NC_    # 2 heads per core
NKT = T // 128       # 16 key tiles per batch
NHT = DFF // 128     # 32 hidden tiles


def _build_nc():
    nc = bacc.Bacc(num_devices=NC_)

    xt_d = nc.dram_tensor("xt", [NBLK, 128, NDC, 512], BF16, kind="ExternalInput")
    xtloc_d = nc.dram_tensor("xt_loc", [128, NDC, 512], F32, kind="ExternalInput")
    wqkv_d = nc.dram_tensor("wqkv", [D, 384], BF16, kind="ExternalInput")
    wsb2_d = nc.dram_tensor("wsb2", [2, 384], F32, kind="ExternalInput")
    wproj_d = nc.dram_tensor("wproj", [128, NDC, D], BF16, kind="ExternalInput")
    w1_d = nc.dram_tensor("w1", [NHT, 128, NDC, 128], BF16, kind="ExternalInput")
    w2_d = nc.dram_tensor("w2", [NDC, 128, NHT, 128], BF16, kind="ExternalInput")
    bproj_d = nc.dram_tensor("bproj_pp", [128, 8], F32, kind="ExternalInput")
    b1_d = nc.dram_tensor("b1_pp", [128, 32], F32, kind="ExternalInput")
    b2_d = nc.dram_tensor("b2_pp", [128, 8], F32, kind="ExternalInput")
    gb1_d = nc.dram_tensor("gb1", [2, D], F32, kind="ExternalInput")
    gb2_d = nc.dram_tensor("gb2", [2, D], F32, kind="ExternalInput")
    masks_d = nc.dram_tensor("masks", [4, 128, 512], BF16, kind="ExternalInput")
    ident_d = nc.dram_tensor("ident", [128, 128], F32, kind="ExternalInput")
    onesd_d = nc.dram_tensor("ones_d", [128, 1], F32, kind="ExternalInput")   # 1/D
    onesdb_d = nc.dram_tensor("ones_db", [128, 1], BF16, kind="ExternalInput")  # 1/D bf16
    ones1_d = nc.dram_tensor("ones_1", [1, 128], F32, kind="ExternalInput")
    out_d = nc.dram_tensor("outT", [D, TOK], F32, kind="ExternalOutput")

    with tile.TileContext(nc) as tc:
        with tc.tile_pool(name="const", bufs=1) as cst:
            def cload(shape, dram_ap, dtype=F32):
                t = cst.tile(shape, dtype, name=f"c{len(nc.m.functions[0].allocations)}")
                nc.gpsimd.dma_start(t[:], dram_ap)
                return t

            wqkv_sb = cload([128, NDC, 384], wqkv_d[:, :].rearrange("(a p) m -> p a m", p=128), BF16)
            masks_sb = cload([128, 4, 512], masks_d[:, :, :].rearrange("m p n -> p m n"), BF16)
            ident_sb = cload([128, 128], ident_d[:, :])
            onesd_sb = cload([128, 1], onesd_d[:, :])
            onesdb_sb = cload([128, 1], onesdb_d[:, :], BF16)
            ones1_sb = cload([1, 128], ones1_d[:, :])
            wsb2_sb = cload([2, 384], wsb2_d[:, :])
            gb1_sb = cload([2, D], gb1_d[:, :])
            gb2_sb = cload([2, D], gb2_d[:, :])
            bproj_sb = cload([128, 8], bproj_d[:, :])
            b1_sb = cload([128, 32], b1_d[:, :])
            b2_sb = cload([128, 8], b2_d[:, :])

            x1n = cst.tile([128, NDC, 512], F32)   # LN2 output, lives into FFN
            x1nb = cst.tile([128, NDC, 512], BF16)  # bf16 copy for W1 matmuls

            # helper: transposed-LN stats -> (A, C) [1, W] tiles at partition 0.
            # src(a) yields the a-th [128, W] d-chunk; sqbuf(a) a scratch tile.
            def tln_stats(sm_pool, ps_pool, W, src, tag):
                s1p = ps_pool.tile([1, W], F32, tag=f"{tag}s1", bufs=1)
                s2p = ps_pool.tile([1, W], F32, tag=f"{tag}s2", bufs=1)
                for a in range(NDC):
                    xa = src(a)
                    sq = sm_pool.tile([128, W], BF16, tag=f"{tag}sq")
                    if a % 2 == 0:
                        nc.scalar.square(sq[:], xa)
                    else:
                        nc.vector.tensor_tensor(sq[:], xa, xa, ALU.mult)
                    nc.tensor.matmul(s1p[:], onesd_sb[:], xa, start=(a == 0), stop=(a == NDC - 1))
                    nc.tensor.matmul(s2p[:], onesdb_sb[:], sq[:], start=(a == 0), stop=(a == NDC - 1))
                mu = sm_pool.tile([1, W], F32, tag=f"{tag}mu", bufs=1)
                nc.scalar.copy(mu[:], s1p[:])
                msq = sm_pool.tile([1, W], F32, tag=f"{tag}msq", bufs=1)
                nc.vector.tensor_tensor(msq[:], mu[:], mu[:], ALU.mult)
                veps = sm_pool.tile([1, W], F32, tag=f"{tag}veps", bufs=1)
                nc.vector.scalar_tensor_tensor(veps[:], s2p[:], EPS, msq[:], ALU.add, ALU.subtract)
                sd = sm_pool.tile([1, W], F32, tag=f"{tag}sd", bufs=1)
                nc.scalar.sqrt(sd[:], veps[:])
                A = sm_pool.tile([1, W], F32, tag=f"{tag}A", bufs=1)
                nc.vector.reciprocal(A[:], sd[:])
                C = sm_pool.tile([2, W], F32, tag=f"{tag}C", bufs=1)
                nc.vector.memset(C[:], 1.0)
                nc.vector.scalar_tensor_tensor(C[0:1, :], mu[:], -1.0, A[:], ALU.mult, ALU.mult)
                return A, C

            # helper: apply x_out = (x*A + C)*g + b on the local slice, chunkwise.
            # A,C are [1,512]; g_row/b_row are [1, D] const rows.
            def tln_apply(sm_pool, ps_pool, A, C, gb_rows, src, dst, tag):
                bal = ps_pool.tile([128, 512], F32, tag=f"{tag}bal", bufs=1)
                nc.tensor.matmul(bal[:], ones1_sb[:], A[:], start=True, stop=True)
                for a in range(NDC):
                    bc = ps_pool.tile([128, 512], F32, tag=f"{tag}bc")
                    nc.tensor.matmul(bc[:], gb_rows[:, 128 * a:128 * (a + 1)], C[:],
                                     start=True, stop=True)
                    tmp = sm_pool.tile([128, 512], F32, tag=f"{tag}tmp")
                    nc.vector.tensor_tensor(tmp[:], src(a), bal[:], ALU.mult)
                    nc.vector.tensor_tensor(dst(a), tmp[:], bc[:], ALU.add)

            with tc.tile_pool(name="warm", bufs=1) as wp, \
                 tc.tile_pool(name="warm_ps", bufs=1, space="PSUM") as wps:
                wt_ = wp.tile([128, 512], BF16)
                nc.vector.memset(wt_[:], 0.001)
                wpt = wps.tile([128, 512], F32)
                for _ in range(36):
                    nc.tensor.matmul(wpt[:], wt_[:, 0:128], wt_[:], start=True, stop=True)

            with tc.tile_pool(name="attn_scope", bufs=1) as big:
                QTh = [big.tile([64, BT], BF16, name=f"qth{i}", tag=f"qt{i}") for i in range(NH_LOC)]
                KTh = [big.tile([64, BT], BF16, name=f"kth{i}", tag=f"kt{i}") for i in range(NH_LOC)]
                VK = big.tile([128, BT], BF16)   # k-major V, both heads side by side
                attnT = big.tile([128, BT], BF16)
                xw = big.tile([128, NDC, 512], F32)   # x_loc -> xn_loc -> x1_loc
                attn_loc = big.tile([128, NDC, 512], BF16)
                nc.sync.dma_start(xw[:], xtloc_d[:, :, :])

                # ===== Phase A: per-block LN1 stats + raw QKV + fixups =====
                with (
                    tc.tile_pool(name="fix", bufs=2) as fxp,
                    tc.tile_pool(name="sweep", bufs=2) as swp,
                    tc.tile_pool(name="sweep_ps", bufs=2, space="PSUM") as swps,
                ):
                    xblks, rows = {}, {}

                    def emit_stats(tb):
                        xblk = xblks[tb]
                        s1p = swps.tile([1, 512], F32, tag="s1p", bufs=1)
                        s2p = swps.tile([1, 512], F32, tag="s2p", bufs=1)
                        for a in range(NDC):
                            sq = swp.tile([128, 512], BF16, tag="sq")
                            if a % 2 == 0:
                                nc.scalar.square(sq[:], xblk[:, a, :])
                            else:
                                nc.vector.tensor_tensor(sq[:], xblk[:, a, :],
                                                        xblk[:, a, :], ALU.mult)
                            nc.tensor.matmul(s1p[:], onesdb_sb[:], xblk[:, a, :],
                                             start=(a == 0), stop=(a == NDC - 1))
                            nc.tensor.matmul(s2p[:], onesdb_sb[:], sq[:],
                                             start=(a == 0), stop=(a == NDC - 1))
                        mu = fxp.tile([1, 512], F32, tag="mu", bufs=2)
                        nc.scalar.copy(mu[:], s1p[:])
                        msq = fxp.tile([1, 512], F32, tag="msq", bufs=2)
                        nc.vector.tensor_tensor(msq[:], mu[:], mu[:], ALU.mult)
                        veps = fxp.tile([1, 512], F32, tag="veps", bufs=2)
                        nc.vector.scalar_tensor_tensor(veps[:], s2p[:], EPS, msq[:],
                                                       ALU.add, ALU.subtract)
                        sd = fxp.tile([1, 512], F32, tag="sd", bufs=2)
                        nc.scalar.sqrt(sd[:], veps[:])
                        arow = fxp.tile([1, 512], F32, tag="arow", bufs=2)
                        nc.vector.reciprocal(arow[:], sd[:])
                        crow = fxp.tile([2, 512], F32, tag="crow", bufs=2)
                        nc.vector.memset(crow[:], 1.0)
                        nc.vector.scalar_tensor_tensor(crow[0:1, :], mu[:], -1.0, arow[:],
                                                       ALU.mult, ALU.mult)
                        rows[tb] = (arow, crow)

                    qk_ps = {}

                    def emit_raw(tb):
                        xblk = swp.tile([128, NDC, 512], BF16, tag="xblk")
                        nc.sync.dma_start(xblk[:], xt_d[tb])
                        xblks[tb] = xblk
                        qkps = []
                        for j in range(3):
                            ps = swps.tile([128, 512], F32, tag="qkvps", bufs=3)
                            for a in range(NDC):
                                nc.tensor.matmul(ps[:], wqkv_sb[:, a, 128 * j:128 * (j + 1)],
                                                 xblk[:, a, :],
                                                 start=(a == 0), stop=(a == NDC - 1))
                            qkps.append(ps)
                        qk_ps[tb] = qkps

                    def emit_fix(tb):
                        ts_ = slice(512 * tb, 512 * (tb + 1))
                        xblks.pop(tb)
                        qkps = qk_ps.pop(tb)
                        arow, crow = rows.pop(tb)
                        ba = swps.tile([128, 512], F32, tag="ba", bufs=2)
                        nc.tensor.matmul(ba[:], ones1_sb[:], arow[:], start=True, stop=True)
                        ba_sb = fxp.tile([128, 512], F32, tag="ba_sb", bufs=1)
                        nc.vector.tensor_copy(ba_sb[:], ba[:])
                        vtr = None
                        for j in range(3):
                            ps = qkps[j]
                            if j < 2:
                                for hh in range(NH_LOC):
                                    co = 128 * j + 64 * hh
                                    f = swps.tile([64, 512], F32, tag="f", bufs=1)
                                    nc.tensor.matmul(f[:], wsb2_sb[:, co:co + 64],
                                                     crow[:], start=True, stop=True)
                                    dst = (QTh if j == 0 else KTh)[hh][:, ts_]
                                    po = 64 * hh
                                    tmp = fxp.tile([64, 512], F32, tag="tmph")
                                    nc.vector.tensor_tensor(tmp[:], ps[po:po + 64, :],
                                                            ba_sb[0:64, :], ALU.mult)
                                    nc.vector.tensor_tensor(dst, tmp[:], f[:], ALU.add)
                            else:
                                f = swps.tile([128, 512], F32, tag="f", bufs=1)
                                nc.tensor.matmul(f[:], wsb2_sb[:, 256:384], crow[:],
                                                 start=True, stop=True)
                                vtr = fxp.tile([128, 512], F32, tag="vtr")
                                tmp = fxp.tile([128, 512], F32, tag="tmp")
                                nc.vector.tensor_tensor(tmp[:], ps[:], ba_sb[:], ALU.mult)
                                nc.vector.tensor_tensor(vtr[:], tmp[:], f[:], ALU.add)
                        for q in range(4):
                            tp = swps.tile([128, 128], F32, tag="ba", bufs=2)
                            nc.tensor.transpose(tp[:], vtr[:, 128 * q:128 * (q + 1)],
                                                ident_sb[:])
                            nc.vector.tensor_copy(
                                VK[:, 512 * tb + 128 * q:512 * tb + 128 * (q + 1)], tp[:])

                    for tb in range(NBLK):
                        emit_raw(tb)
                        emit_stats(tb)
                        emit_fix(tb)

                # ===== Phase B: attention per (batch, local head), A2A per batch =====
                with (
                    tc.tile_pool(name="se_pool", bufs=1) as sep,
                    tc.tile_pool(name="attn_sm", bufs=2) as asm,
                    tc.tile_pool(name="attn_ps", bufs=2, space="PSUM") as aps,
                    tc.tile_pool(name="dram", bufs=1, space="DRAM") as dpool,
                ):
                    for b in range(B):
                        # fused heads: St/exp for kt (both heads), col-packed
                        # attnT MMs for kt-1 (h0 -> psum rows 0:64, h1 -> 64:128)
                        se_tiles, vc_tiles = {}, {}
                        ap_tiles = [aps.tile([128, 512], F32, name=f"app{qb}",
                                             tag=f"ap{qb}", bufs=1)
                                    for qb in range(4)]

                        def emit_st(kt, hh):
                            QT, KT = QTh[hh], KTh[hh]
                            qb0 = (kt * 128) // 512
                            nqb = 4 - qb0
                            se = sep.tile([128, 512 * nqb], BF16, name=f"se_{kt}_{hh}",
                                          tag=f"se{hh}", bufs=3)
                            se_tiles[(kt, hh)] = se
                            cparts = asm.tile([128, 4], F32, tag="cparts")
                            for qb in range(qb0, 4):
                                stp = aps.tile([128, 512], F32, tag="stp", bufs=4)
                                nc.tensor.matmul(
                                    stp[:],
                                    KT[:, b * T + 128 * kt: b * T + 128 * (kt + 1)],
                                    QT[:, b * T + 512 * qb: b * T + 512 * (qb + 1)],
                                    start=True, stop=True)
                                col = 512 * (qb - qb0)
                                i = qb - qb0
                                if qb == qb0:
                                    et = asm.tile([128, 512], BF16, tag="et", bufs=3)
                                    nc.scalar.activation(et[:], stp[:], AF.Exp)
                                    nc.vector.scalar_tensor_tensor(
                                        se[:, col:col + 512], et[:], 1.0,
                                        masks_sb[:, kt % 4, :], ALU.mult, ALU.mult,
                                        accum_out=cparts[:, i:i + 1])
                                else:
                                    nc.scalar.activation(
                                        se[:, col:col + 512], stp[:], AF.Exp,
                                        accum_out=cparts[:, i:i + 1])
                            ck = asm.tile([128, 1], F32, tag="ck", bufs=3)
                            nc.vector.tensor_reduce(ck[:], cparts[:, 0:nqb],
                                                    mybir.AxisListType.X, ALU.add)
                            rk = asm.tile([128, 1], F32, tag="rk", bufs=3)
                            nc.vector.reciprocal(rk[:], ck[:])
                            vc = asm.tile([128, 64], BF16, tag=f"vc{hh}", bufs=3)
                            ktf = b * NKT + kt
                            nc.vector.tensor_scalar_mul(
                                vc[:], VK[:, 128 * ktf + 64 * hh:128 * ktf + 64 * hh + 64],
                                rk[:])
                            vc_tiles[(kt, hh)] = vc

                        def emit_at(kt):
                            qb0 = (kt * 128) // 512
                            for qb in range(qb0, 4):
                                last = (kt == 4 * (qb + 1) - 1)
                                for hh in range(NH_LOC):
                                    nc.tensor.matmul(
                                        ap_tiles[qb][64 * hh:64 * hh + 64, :],
                                        vc_tiles[(kt, hh)][:],
                                        se_tiles[(kt, hh)][:, 512 * (qb - qb0):
                                                           512 * (qb - qb0) + 512],
                                        start=(kt == 0), stop=last,
                                        tile_position=(0, 64 * hh),
                                        skip_group_check=True)
                                if last:
                                    nc.vector.tensor_copy(
                                        attnT[:, b * T + 512 * qb: b * T + 512 * (qb + 1)],
                                        ap_tiles[qb][:])

                        for kt in range(NKT + 1):
                            if kt < NKT:
                                emit_st(kt, 0)
                                emit_st(kt, 1)
                            if kt >= 1:
                                emit_at(kt - 1)
                        # AllToAll for this batch (overlaps the next batch's compute):
                        # core c owns tokens [256c, 256(c+1)) of EACH batch.
                        a2a_in = dpool.tile([8, 128, 256], BF16, name=f"a2ai{b}", tag=f"a2ai{b}")
                        a2a_out = dpool.tile([8, 128, 256], BF16, name=f"a2ao{b}", tag=f"a2ao{b}")
                        for j in range(8):
                            nc.sync.dma_start(a2a_in[j],
                                              attnT[:, b * T + 256 * j:b * T + 256 * (j + 1)])
                        nc.gpsimd.collective_compute(
                            "AllToAll", ALU.bypass,
                            replica_groups=[list(range(NC_))],
                            ins=[a2a_in.opt()], outs=[a2a_out.opt()])
                        for s in range(8):
                            nc.sync.dma_start(attn_loc[:, s, 256 * b:256 * (b + 1)], a2a_out[s])

                # ---- LN1 apply on the local slice (xw := xn_loc) ----
                with (
                    tc.tile_pool(name="l1_sm", bufs=2) as l1sm,
                    tc.tile_pool(name="l1_ps", bufs=2, space="PSUM") as l1ps,
                ):
                    myA, myC = tln_stats(l1sm, l1ps, 512, lambda a: xw[:, a, :], "l1")
                    tln_apply(l1sm, l1ps, myA, myC, gb1_sb,
                              lambda a: xw[:, a, :], lambda a: xw[:, a, :], "l1")

                # ===== Phase D: projection + residual + LN2 (local 512 tokens) =====
                with (
                    tc.tile_pool(name="proj_w", bufs=1) as pjw,
                    tc.tile_pool(name="proj_sm", bufs=2) as pjm,
                    tc.tile_pool(name="proj_ps", bufs=2, space="PSUM") as pjps,
                ):
                    wproj_sb = pjw.tile([128, NDC, D], BF16)
                    nc.sync.dma_start(wproj_sb[:], wproj_d[:, :, :])
                    for dt in range(NDC):
                        pp = pjps.tile([128, 512], F32, tag="pp")
                        for a in range(NDC):
                            nc.tensor.matmul(pp[:], wproj_sb[:, a, 128 * dt:128 * (dt + 1)],
                                             attn_loc[:, a, :],
                                             start=(a == 0), stop=(a == NDC - 1))
                        nc.vector.scalar_tensor_tensor(
                            xw[:, dt, :], pp[:], bproj_sb[:, dt:dt + 1], xw[:, dt, :],
                            ALU.add, ALU.add)
                    A2, C2 = tln_stats(pjm, pjps, 512, lambda a: xw[:, a, :], "l2")
                    tln_apply(pjm, pjps, A2, C2, gb2_sb,
                              lambda a: xw[:, a, :], lambda a: x1n[:, a, :], "l2")
                    for a in range(NDC):
                        nc.vector.tensor_copy(x1nb[:, a, :], x1n[:, a, :])

            # ===== Phase E: FFN (token-sharded, streamed weights) =====
            with (
                tc.tile_pool(name="ffn_h", bufs=1) as fb,
                tc.tile_pool(name="ffn_w1", bufs=3) as w1p,
                tc.tile_pool(name="ffn_w2", bufs=2) as w2p,
                tc.tile_pool(name="ffn_sm", bufs=2) as fsm,
                tc.tile_pool(name="ffn_ps", bufs=2, space="PSUM") as fps,
            ):
                hT = fb.tile([128, NHT, 512], BF16)
                for ht in range(NHT):
                    w1t = w1p.tile([128, NDC, 128], BF16, tag="w1")
                    nc.sync.dma_start(w1t[:], w1_d[ht])
                    hp = fps.tile([128, 512], F32, tag="hp")
                    for a in range(NDC):
                        nc.tensor.matmul(hp[:], w1t[:, a, :], x1nb[:, a, :],
                                         start=(a == 0), stop=(a == NDC - 1))
                    nc.scalar.activation(hT[:, ht, :], hp[:], AF.Relu, bias=b1_sb[:, ht:ht + 1])
                for dt in range(NDC):
                    w2t = w2p.tile([128, NHT, 128], BF16, tag="w2")
                    nc.sync.dma_start(w2t[:], w2_d[dt])
                    fp_ = fps.tile([128, 512], F32, tag="fp")
                    for a2_ in range(NHT):
                        nc.tensor.matmul(fp_[:], w2t[:, a2_, :], hT[:, a2_, :],
                                         start=(a2_ == 0), stop=(a2_ == NHT - 1))
                    ot = fsm.tile([128, 512], F32, tag="ot")
                    nc.vector.scalar_tensor_tensor(ot[:], fp_[:], b2_sb[:, dt:dt + 1],
                                                   x1n[:, dt, :], ALU.add, ALU.add)
                    nc.sync.dma_start(
                        out_d[:, :].rearrange("(a p) n -> p a n", p=128)[:, dt, :], ot[:])
    nc.compile()
    return nc


_NC_CACHE = None


def _get_nc():
    global _NC_CACHE
    if _NC_CACHE is None:
        _NC_CACHE = _build_nc()
    return _NC_CACHE


def make_in_maps(inputs):
    x = np.asarray(inputs["x"], np.float32)
    Wq = np.asarray(inputs["Wq"], np.float32)
    Wk = np.asarray(inputs["Wk"], np.float32)
    Wv = np.asarray(inputs["Wv"], np.float32)
    Wproj = np.ascontiguousarray(np.asarray(inputs["Wproj"], np.float32))
    bproj = np.asarray(inputs["bproj"], np.float32)
    W1 = np.ascontiguousarray(np.asarray(inputs["W1"], np.float32))
    b1 = np.asarray(inputs["b1"], np.float32)
    W2 = np.ascontiguousarray(np.asarray(inputs["W2"], np.float32))
    b2 = np.asarray(inputs["b2"], np.float32)
    g1 = np.asarray(inputs["ln1_g"], np.float32)
    bl1 = np.asarray(inputs["ln1_b"], np.float32)
    g2 = np.asarray(inputs["ln2_g"], np.float32)
    bl2 = np.asarray(inputs["ln2_b"], np.float32)

    s = np.float32(D ** -0.5)
    # xt tiled: [NBLK, 128, NDC, 512]: block tb = flat tokens 512tb..512(tb+1),
    # chunk a = features 128a..128(a+1)
    x_flat = x.reshape(BT, D)
    xt = np.ascontiguousarray(
        x_flat.reshape(NBLK, 512, NDC, 128).transpose(0, 3, 2, 1)
    ).astype(ml_dtypes.bfloat16)

    masks = np.zeros((4, 128, 512), np.float32)
    jj = np.arange(512)[None, :]
    ii = np.arange(128)[:, None]
    for m in range(4):
        masks[m] = (jj >= m * 128 + ii).astype(np.float32)
    masks = masks.astype(ml_dtypes.bfloat16)

    common = {
        "xt": xt,
        "wproj": np.ascontiguousarray(Wproj.reshape(NDC, 128, D).transpose(1, 0, 2)).astype(ml_dtypes.bfloat16),
        "w1": np.ascontiguousarray(W1.reshape(NDC, 128, NHT, 128).transpose(2, 1, 0, 3)).astype(ml_dtypes.bfloat16),
        "w2": np.ascontiguousarray(W2.reshape(NHT, 128, NDC, 128).transpose(2, 1, 0, 3)).astype(ml_dtypes.bfloat16),
        "bproj_pp": np.ascontiguousarray(bproj.reshape(8, 128).T),
        "b1_pp": np.ascontiguousarray(b1.reshape(32, 128).T),
        "b2_pp": np.ascontiguousarray(b2.reshape(8, 128).T),
        "gb1": np.ascontiguousarray(np.stack([g1, bl1])),
        "gb2": np.ascontiguousarray(np.stack([g2, bl2])),
        "masks": masks,
        "ident": np.eye(128, dtype=np.float32),
        "ones_d": np.full((128, 1), 1.0 / D, np.float32),
        "ones_db": np.full((128, 1), 1.0 / D, ml_dtypes.bfloat16),
        "ones_1": np.ones((1, 128), np.float32),
    }

    in_maps = []
    for c in range(NC_):
        h0 = NH_LOC * c
        Wq_cat = np.concatenate([Wq[h0 + i] for i in range(NH_LOC)], 1)  # [D,128]
        Wk_cat = np.concatenate([Wk[h0 + i] for i in range(NH_LOC)], 1)
        Wv_cat = np.concatenate([Wv[h0 + i] for i in range(NH_LOC)], 1)
        Wq_eff = g1[:, None] * Wq_cat * s
        Wk_eff = g1[:, None] * Wk_cat
        Wv_eff = g1[:, None] * Wv_cat
        wqkv = np.ascontiguousarray(np.concatenate([Wq_eff, Wk_eff, Wv_eff], 1)).astype(ml_dtypes.bfloat16)
        wsums = np.concatenate([Wq_eff.sum(0), Wk_eff.sum(0), Wv_eff.sum(0)])
        wbias = np.concatenate([bl1 @ (Wq_cat * s), bl1 @ Wk_cat, bl1 @ Wv_cat])
        m = dict(common)
        xl = np.concatenate([x[0, 256 * c:256 * (c + 1)],
                             x[1, 256 * c:256 * (c + 1)]], axis=0)  # [512, D]
        m["xt_loc"] = np.ascontiguousarray(xl.reshape(512, NDC, 128).transpose(2, 1, 0))
        m["wqkv"] = wqkv
        m["wsb2"] = np.ascontiguousarray(
            np.stack([wsums, wbias]).astype(np.float32))
        in_maps.append(m)
    return in_maps


def run(inputs, trace=False, trace_kwargs=None):
    nc = _get_nc()
    in_maps = make_in_maps(inputs)
    res = run_bass_kernel_spmd(nc, in_maps, core_ids=list(range(NC_)),
                               trace=trace, **(trace_kwargs or {}))
    out = np.empty((B, T, D), np.float32)
    for c in range(NC_):
        o = res.results[c]["outT"]
        out[0, 256 * c:256 * (c + 1)] = o[:, 0:256].T
        out[1, 256 * c:256 * (c + 1)] = o[:, 256:512].T
    return out, res


def kernel(**inputs) -> np.ndarray:
    out, _ = run(inputs, trace=False)
    return out


# revision 31
# speedup vs baseline: 1.1449x; 1.1449x over previous
"""Trainium2 Bass/Tile kernel for a dense transformer block (B=2, T=2048, D=1024, H=16).

Sharding across 8 NeuronCores, designed to avoid large collectives:
  - LayerNorm1 statistics are computed (replicated) from the transposed input
    via PE-matmul column reductions; the LN *apply* is algebraically folded
    into each core's Q/K/V projections (X = A*X_raw + C*wsum + wb).
  - Attention is head-sharded: each core owns 2 of the 16 heads for both
    batches. The reference's query-axis softmax is reformulated as
      attn^T = (V/c)^T @ exp(S^T),  c[k] = row-sum of exp(S^T)
    so the normalizer is a cheap free-axis reduction folded into V.
  - One 2MB/core AllToAll reshards attention output from head-split to
    token-split (512 tokens per core).
  - Projection, LN2 and the FFN run token-sharded with replicated, streamed
    W1/W2 weights.
Everything is computed in feature-major ("transposed") layout so every matmul
has its natural lhsT operand; the host supplies x^T and re-transposes the
512-token output slices at the end.
"""

import numpy as np
import ml_dtypes

import concourse.bass as bass
import concourse.bacc as bacc
import concourse.mybir as mybir
import concourse.tile as tile
from concourse.bass_utils import run_bass_kernel_spmd

F32 = mybir.dt.float32
BF16 = mybir.dt.bfloat16
AF = mybir.ActivationFunctionType
ALU = mybir.AluOpType

B, T, D, H = 2, 2048, 1024, 16
HS = D // H          # 64
DFF = 4 * D          # 4096
EPS = 1e-5
NC_ = 8              # cores
BT = B * T           # 4096 flat tokens
TOK = BT // NC_      # 512 tokens per core
NBLK = BT // 512     # 8 token blocks
NDC = D // 128       # 8 d-chunks
NH_LOC = H // NC_    # 2 heads per core
NKT = T // 128       # 16 key tiles per batch
NHT = DFF // 128     # 32 hidden tiles


def _build_nc():
    nc = bacc.Bacc(num_devices=NC_)

    xt_d = nc.dram_tensor("xt", [NBLK, 128, NDC, 512], BF16, kind="ExternalInput")
    xtloc_d = nc.dram_tensor("xt_loc", [128, NDC, 512], F32, kind="ExternalInput")
    wqkv_d = nc.dram_tensor("wqkv", [D, 384], BF16, kind="ExternalInput")
    wsb2_d = nc.dram_tensor("wsb2", [2, 384], F32, kind="ExternalInput")
    wproj_d = nc.dram_tensor("wproj", [128, NDC, D], BF16, kind="ExternalInput")
    w1_d = nc.dram_tensor("w1", [NHT, 128, NDC, 128], BF16, kind="ExternalInput")
    w2_d = nc.dram_tensor("w2", [NDC, 128, NHT, 128], BF16, kind="ExternalInput")
    bproj_d = nc.dram_tensor("bproj_pp", [128, 8], F32, kind="ExternalInput")
    b1_d = nc.dram_tensor("b1_pp", [128, 32], F32, kind="ExternalInput")
    b2_d = nc.dram_tensor("b2_pp", [128, 8], F32, kind="ExternalInput")
    gb1_d = nc.dram_tensor("gb1", [2, D], F32, kind="ExternalInput")
    gb2_d = nc.dram_tensor("gb2", [2, D], F32, kind="ExternalInput")
    masks_d = nc.dram_tensor("masks", [4, 128, 512], BF16, kind="ExternalInput")
    ident_d = nc.dram_tensor("ident", [128, 128], F32, kind="ExternalInput")
    onesd_d = nc.dram_tensor("ones_d", [128, 1], F32, kind="ExternalInput")   # 1/D
    onesdb_d = nc.dram_tensor("ones_db", [128, 1], BF16, kind="ExternalInput")  # 1/D bf16
    ones1_d = nc.dram_tensor("ones_1", [1, 128], F32, kind="ExternalInput")
    out_d = nc.dram_tensor("outT", [D, TOK], F32, kind="ExternalOutput")

    with tile.TileContext(nc) as tc:
        with tc.tile_pool(name="const", bufs=1) as cst:
            def cload(shape, dram_ap, dtype=F32):
                t = cst.tile(shape, dtype, name=f"c{len(nc.m.functions[0].allocations)}")
                nc.gpsimd.dma_start(t[:], dram_ap)
                return t

            wqkv_sb = cload([128, NDC, 384], wqkv_d[:, :].rearrange("(a p) m -> p a m", p=128), BF16)
            masks_sb = cload([128, 4, 512], masks_d[:, :, :].rearrange("m p n -> p m n"), BF16)
            ident_sb = cload([128, 128], ident_d[:, :])
            onesd_sb = cload([128, 1], onesd_d[:, :])
            onesdb_sb = cload([128, 1], onesdb_d[:, :], BF16)
            ones1_sb = cload([1, 128], ones1_d[:, :])
            wsb2_sb = cload([2, 384], wsb2_d[:, :])
            gb1_sb = cload([2, D], gb1_d[:, :])
            gb2_sb = cload([2, D], gb2_d[:, :])
            bproj_sb = cload([128, 8], bproj_d[:, :])
            b1_sb = cload([128, 32], b1_d[:, :])
            b2_sb = cload([128, 8], b2_d[:, :])

            x1n = cst.tile([128, NDC, 512], F32)   # LN2 output, lives into FFN
            x1nb = cst.tile([128, NDC, 512], BF16)  # bf16 copy for W1 matmuls

            # helper: transposed-LN stats -> (A, C) [1, W] tiles at partition 0.
            # src(a) yields the a-th [128, W] d-chunk; sqbuf(a) a scratch tile.
            def tln_stats(sm_pool, ps_pool, W, src, tag):
                s1p = ps_pool.tile([1, W], F32, tag=f"{tag}s1", bufs=1)
                s2p = ps_pool.tile([1, W], F32, tag=f"{tag}s2", bufs=1)
                for a in range(NDC):
                    xa = src(a)
                    sq = sm_pool.tile([128, W], BF16, tag=f"{tag}sq")
                    nc.scalar.square(sq[:], xa)
                    nc.tensor.matmul(s1p[:], onesd_sb[:], xa, start=(a == 0), stop=(a == NDC - 1))
                    nc.tensor.matmul(s2p[:], onesdb_sb[:], sq[:], start=(a == 0), stop=(a == NDC - 1))
                mu = sm_pool.tile([1, W], F32, tag=f"{tag}mu", bufs=1)
                nc.scalar.copy(mu[:], s1p[:])
                msq = sm_pool.tile([1, W], F32, tag=f"{tag}msq", bufs=1)
                nc.vector.tensor_tensor(msq[:], mu[:], mu[:], ALU.mult)
                veps = sm_pool.tile([1, W], F32, tag=f"{tag}veps", bufs=1)
                nc.vector.scalar_tensor_tensor(veps[:], s2p[:], EPS, msq[:], ALU.add, ALU.subtract)
                sd = sm_pool.tile([1, W], F32, tag=f"{tag}sd", bufs=1)
                nc.scalar.sqrt(sd[:], veps[:])
                A = sm_pool.tile([1, W], F32, tag=f"{tag}A", bufs=1)
                nc.vector.reciprocal(A[:], sd[:])
                C = sm_pool.tile([2, W], F32, tag=f"{tag}C", bufs=1)
                nc.vector.memset(C[:], 1.0)
                nc.vector.scalar_tensor_tensor(C[0:1, :], mu[:], -1.0, A[:], ALU.mult, ALU.mult)
                return A, C

            # helper: apply x_out = (x*A + C)*g + b on the local slice, chunkwise.
            # A,C are [1,512]; g_row/b_row are [1, D] const rows.
            def tln_apply(sm_pool, ps_pool, A, C, gb_rows, src, dst, tag):
                bal = ps_pool.tile([128, 512], F32, tag=f"{tag}bal", bufs=1)
                nc.tensor.matmul(bal[:], ones1_sb[:], A[:], start=True, stop=True)
                for a in range(NDC):
                    bc = ps_pool.tile([128, 512], F32, tag=f"{tag}bc")
                    nc.tensor.matmul(bc[:], gb_rows[:, 128 * a:128 * (a + 1)], C[:],
                                     start=True, stop=True)
                    tmp = sm_pool.tile([128, 512], F32, tag=f"{tag}tmp")
                    nc.vector.tensor_tensor(tmp[:], src(a), bal[:], ALU.mult)
                    nc.vector.tensor_tensor(dst(a), tmp[:], bc[:], ALU.add)

            with tc.tile_pool(name="warm", bufs=1) as wp, \
                 tc.tile_pool(name="warm_ps", bufs=1, space="PSUM") as wps:
                wt_ = wp.tile([128, 512], BF16)
                nc.vector.memset(wt_[:], 0.001)
                wpt = wps.tile([128, 512], F32)
                for _ in range(36):
                    nc.tensor.matmul(wpt[:], wt_[:, 0:128], wt_[:], start=True, stop=True)

            with tc.tile_pool(name="attn_scope", bufs=1) as big:
                QTh = [big.tile([64, BT], BF16, name=f"qth{i}", tag=f"qt{i}") for i in range(NH_LOC)]
                KTh = [big.tile([64, BT], BF16, name=f"kth{i}", tag=f"kt{i}") for i in range(NH_LOC)]
                VK = big.tile([128, BT], BF16)   # k-major V, both heads side by side
                attnT = big.tile([128, BT], BF16)
                xw = big.tile([128, NDC, 512], F32)   # x_loc -> xn_loc -> x1_loc
                attn_loc = big.tile([128, NDC, 512], BF16)
                nc.sync.dma_start(xw[:], xtloc_d[:, :, :])

                # ===== Phase A: per-block LN1 stats + raw QKV + fixups =====
                with (
                    tc.tile_pool(name="fix", bufs=2) as fxp,
                    tc.tile_pool(name="sweep", bufs=2) as swp,
                    tc.tile_pool(name="sweep_ps", bufs=2, space="PSUM") as swps,
                ):
                    xblks, rows = {}, {}

                    def emit_stats(tb):
                        xblk = xblks[tb]
                        s1p = swps.tile([1, 512], F32, tag="s1p", bufs=1)
                        s2p = swps.tile([1, 512], F32, tag="s2p", bufs=1)
                        for a in range(NDC):
                            sq = swp.tile([128, 512], BF16, tag="sq")
                            if a % 2 == 0:
                                nc.scalar.square(sq[:], xblk[:, a, :])
                            else:
                                nc.vector.tensor_tensor(sq[:], xblk[:, a, :],
                                                        xblk[:, a, :], ALU.mult)
                            nc.tensor.matmul(s1p[:], onesdb_sb[:], xblk[:, a, :],
                                             start=(a == 0), stop=(a == NDC - 1))
                            nc.tensor.matmul(s2p[:], onesdb_sb[:], sq[:],
                                             start=(a == 0), stop=(a == NDC - 1))
                        mu = fxp.tile([1, 512], F32, tag="mu", bufs=2)
                        nc.scalar.copy(mu[:], s1p[:])
                        msq = fxp.tile([1, 512], F32, tag="msq", bufs=2)
                        nc.vector.tensor_tensor(msq[:], mu[:], mu[:], ALU.mult)
                        veps = fxp.tile([1, 512], F32, tag="veps", bufs=2)
                        nc.vector.scalar_tensor_tensor(veps[:], s2p[:], EPS, msq[:],
                                                       ALU.add, ALU.subtract)
                        sd = fxp.tile([1, 512], F32, tag="sd", bufs=2)
                        nc.scalar.sqrt(sd[:], veps[:])
                        arow = fxp.tile([1, 512], F32, tag="arow", bufs=2)
                        nc.vector.reciprocal(arow[:], sd[:])
                        crow = fxp.tile([2, 512], F32, tag="crow", bufs=2)
                        nc.vector.memset(crow[:], 1.0)
                        nc.vector.scalar_tensor_tensor(crow[0:1, :], mu[:], -1.0, arow[:],
                                                       ALU.mult, ALU.mult)
                        rows[tb] = (arow, crow)

                    qk_ps = {}

                    def emit_raw(tb):
                        xblk = swp.tile([128, NDC, 512], BF16, tag="xblk")
                        nc.sync.dma_start(xblk[:], xt_d[tb])
                        xblks[tb] = xblk
                        qkps = []
                        for j in range(3):
                            ps = swps.tile([128, 512], F32, tag="qkvps", bufs=3)
                            for a in range(NDC):
                                nc.tensor.matmul(ps[:], wqkv_sb[:, a, 128 * j:128 * (j + 1)],
                                                 xblk[:, a, :],
                                                 start=(a == 0), stop=(a == NDC - 1))
                            qkps.append(ps)
                        qk_ps[tb] = qkps

                    def emit_fix(tb):
                        ts_ = slice(512 * tb, 512 * (tb + 1))
                        xblks.pop(tb)
                        qkps = qk_ps.pop(tb)
                        arow, crow = rows.pop(tb)
                        ba = swps.tile([128, 512], F32, tag="ba", bufs=2)
                        nc.tensor.matmul(ba[:], ones1_sb[:], arow[:], start=True, stop=True)
                        ba_sb = fxp.tile([128, 512], F32, tag="ba_sb", bufs=1)
                        nc.vector.tensor_copy(ba_sb[:], ba[:])
                        vtr = None
                        for j in range(3):
                            ps = qkps[j]
                            if j < 2:
                                for hh in range(NH_LOC):
                                    co = 128 * j + 64 * hh
                                    f = swps.tile([64, 512], F32, tag="f", bufs=1)
                                    nc.tensor.matmul(f[:], wsb2_sb[:, co:co + 64],
                                                     crow[:], start=True, stop=True)
                                    dst = (QTh if j == 0 else KTh)[hh][:, ts_]
                                    po = 64 * hh
                                    tmp = fxp.tile([64, 512], F32, tag="tmph")
                                    nc.vector.tensor_tensor(tmp[:], ps[po:po + 64, :],
                                                            ba_sb[0:64, :], ALU.mult)
                                    nc.vector.tensor_tensor(dst, tmp[:], f[:], ALU.add)
                            else:
                                f = swps.tile([128, 512], F32, tag="f", bufs=1)
                                nc.tensor.matmul(f[:], wsb2_sb[:, 256:384], crow[:],
                                                 start=True, stop=True)
                                vtr = fxp.tile([128, 512], F32, tag="vtr")
                                tmp = fxp.tile([128, 512], F32, tag="tmp")
                                nc.vector.tensor_tensor(tmp[:], ps[:], ba_sb[:], ALU.mult)
                                nc.vector.tensor_tensor(vtr[:], tmp[:], f[:], ALU.add)
                        for q in range(4):
                            tp = swps.tile([128, 128], F32, tag="ba", bufs=2)
                            nc.tensor.transpose(tp[:], vtr[:, 128 * q:128 * (q + 1)],
                                                ident_sb[:])
                            nc.vector.tensor_copy(
                                VK[:, 512 * tb + 128 * q:512 * tb + 128 * (q + 1)], tp[:])

                    for tb in range(NBLK):
                        emit_raw(tb)
                        emit_stats(tb)
                        emit_fix(tb)

                # ===== Phase B: attention per (batch, local head), A2A per batch =====
                with (
                    tc.tile_pool(name="se_pool", bufs=1) as sep,
                    tc.tile_pool(name="attn_sm", bufs=2) as asm,
                    tc.tile_pool(name="attn_ps", bufs=2, space="PSUM") as aps,
                    tc.tile_pool(name="dram", bufs=1, space="DRAM") as dpool,
                ):
                    for b in range(B):
                        # fused heads: St/exp for kt (both heads), col-packed
                        # attnT MMs for kt-1 (h0 -> psum rows 0:64, h1 -> 64:128)
                        se_tiles, vc_tiles = {}, {}
                        ap_tiles = [aps.tile([128, 512], F32, name=f"app{qb}",
                                             tag=f"ap{qb}", bufs=1)
                                    for qb in range(4)]

                        def emit_st(kt, hh):
                            QT, KT = QTh[hh], KTh[hh]
                            qb0 = (kt * 128) // 512
                            nqb = 4 - qb0
                            se = sep.tile([128, 512 * nqb], BF16, name=f"se_{kt}_{hh}",
                                          tag=f"se{hh}", bufs=3)
                            se_tiles[(kt, hh)] = se
                            cparts = asm.tile([128, 4], F32, tag="cparts")
                            for qb in range(qb0, 4):
                                stp = aps.tile([128, 512], F32, tag="stp", bufs=4)
                                nc.tensor.matmul(
                                    stp[:],
                                    KT[:, b * T + 128 * kt: b * T + 128 * (kt + 1)],
                                    QT[:, b * T + 512 * qb: b * T + 512 * (qb + 1)],
                                    start=True, stop=True)
                                col = 512 * (qb - qb0)
                                i = qb - qb0
                                if qb == qb0:
                                    et = asm.tile([128, 512], BF16, tag="et", bufs=3)
                                    nc.scalar.activation(et[:], stp[:], AF.Exp)
                                    nc.vector.scalar_tensor_tensor(
                                        se[:, col:col + 512], et[:], 1.0,
                                        masks_sb[:, kt % 4, :], ALU.mult, ALU.mult,
                                        accum_out=cparts[:, i:i + 1])
                                else:
                                    nc.scalar.activation(
                                        se[:, col:col + 512], stp[:], AF.Exp,
                                        accum_out=cparts[:, i:i + 1])
                            ck = asm.tile([128, 1], F32, tag="ck", bufs=3)
                            nc.vector.tensor_reduce(ck[:], cparts[:, 0:nqb],
                                                    mybir.AxisListType.X, ALU.add)
                            rk = asm.tile([128, 1], F32, tag="rk", bufs=3)
                            nc.vector.reciprocal(rk[:], ck[:])
                            vc = asm.tile([128, 64], BF16, tag=f"vc{hh}", bufs=3)
                            ktf = b * NKT + kt
                            nc.vector.tensor_scalar_mul(
                                vc[:], VK[:, 128 * ktf + 64 * hh:128 * ktf + 64 * hh + 64],
                                rk[:])
                            vc_tiles[(kt, hh)] = vc

                        def emit_at(kt):
                            qb0 = (kt * 128) // 512
                            for qb in range(qb0, 4):
                                last = (kt == 4 * (qb + 1) - 1)
                                for hh in range(NH_LOC):
                                    nc.tensor.matmul(
                                        ap_tiles[qb][64 * hh:64 * hh + 64, :],
                                        vc_tiles[(kt, hh)][:],
                                        se_tiles[(kt, hh)][:, 512 * (qb - qb0):
                                                           512 * (qb - qb0) + 512],
                                        start=(kt == 0), stop=last,
                                        tile_position=(0, 64 * hh),
                                        skip_group_check=True)
                                if last:
                                    nc.vector.tensor_copy(
                                        attnT[:, b * T + 512 * qb: b * T + 512 * (qb + 1)],
                                        ap_tiles[qb][:])

                        for kt in range(NKT + 1):
                            if kt < NKT:
                                emit_st(kt, 0)
                                emit_st(kt, 1)
                            if kt >= 1:
                                emit_at(kt - 1)
                        # AllToAll for this batch (overlaps the next batch's compute):
                        # core c owns tokens [256c, 256(c+1)) of EACH batch.
                        a2a_in = dpool.tile([8, 128, 256], BF16, name=f"a2ai{b}", tag=f"a2ai{b}")
                        a2a_out = dpool.tile([8, 128, 256], BF16, name=f"a2ao{b}", tag=f"a2ao{b}")
                        for j in range(8):
                            nc.sync.dma_start(a2a_in[j],
                                              attnT[:, b * T + 256 * j:b * T + 256 * (j + 1)])
                        nc.gpsimd.collective_compute(
                            "AllToAll", ALU.bypass,
                            replica_groups=[list(range(NC_))],
                            ins=[a2a_in.opt()], outs=[a2a_out.opt()])
                        for s in range(8):
                            nc.sync.dma_start(attn_loc[:, s, 256 * b:256 * (b + 1)], a2a_out[s])

                # ---- LN1 apply on the local slice (xw := xn_loc) ----
                with (
                    tc.tile_pool(name="l1_sm", bufs=2) as l1sm,
                    tc.tile_pool(name="l1_ps", bufs=2, space="PSUM") as l1ps,
                ):
                    myA, myC = tln_stats(l1sm, l1ps, 512, lambda a: xw[:, a, :], "l1")
                    tln_apply(l1sm, l1ps, myA, myC, gb1_sb,
                              lambda a: xw[:, a, :], lambda a: xw[:, a, :], "l1")

                # ===== Phase D: projection + residual + LN2 (local 512 tokens) =====
                with (
                    tc.tile_pool(name="proj_w", bufs=1) as pjw,
                    tc.tile_pool(name="proj_sm", bufs=2) as pjm,
                    tc.tile_pool(name="proj_ps", bufs=2, space="PSUM") as pjps,
                ):
                    wproj_sb = pjw.tile([128, NDC, D], BF16)
                    nc.sync.dma_start(wproj_sb[:], wproj_d[:, :, :])
                    for dt in range(NDC):
                        pp = pjps.tile([128, 512], F32, tag="pp")
                        for a in range(NDC):
                            nc.tensor.matmul(pp[:], wproj_sb[:, a, 128 * dt:128 * (dt + 1)],
                                             attn_loc[:, a, :],
                                             start=(a == 0), stop=(a == NDC - 1))
                        nc.vector.scalar_tensor_tensor(
                            xw[:, dt, :], pp[:], bproj_sb[:, dt:dt + 1], xw[:, dt, :],
                            ALU.add, ALU.add)
                    A2, C2 = tln_stats(pjm, pjps, 512, lambda a: xw[:, a, :], "l2")
                    tln_apply(pjm, pjps, A2, C2, gb2_sb,
                              lambda a: xw[:, a, :], lambda a: x1n[:, a, :], "l2")
                    for a in range(NDC):
                        nc.vector.tensor_copy(x1nb[:, a, :], x1n[:, a, :])

            # ===== Phase E: FFN (token-sharded, streamed weights) =====
            with (
                tc.tile_pool(name="ffn_h", bufs=1) as fb,
                tc.tile_pool(name="ffn_w1", bufs=3) as w1p,
                tc.tile_pool(name="ffn_w2", bufs=2) as w2p,
                tc.tile_pool(name="ffn_sm", bufs=2) as fsm,
                tc.tile_pool(name="ffn_ps", bufs=2, space="PSUM") as fps,
            ):
                hT = fb.tile([128, NHT, 512], BF16)
                for ht in range(NHT):
                    w1t = w1p.tile([128, NDC, 128], BF16, tag="w1")
                    nc.sync.dma_start(w1t[:], w1_d[ht])
                    hp = fps.tile([128, 512], F32, tag="hp")
                    for a in range(NDC):
                        nc.tensor.matmul(hp[:], w1t[:, a, :], x1nb[:, a, :],
                                         start=(a == 0), stop=(a == NDC - 1))
                    nc.scalar.activation(hT[:, ht, :], hp[:], AF.Relu, bias=b1_sb[:, ht:ht + 1])
                for dt in range(NDC):
                    w2t = w2p.tile([128, NHT, 128], BF16, tag="w2")
                    nc.sync.dma_start(w2t[:], w2_d[dt])
                    fp_ = fps.tile([128, 512], F32, tag="fp")
                    for a2_ in range(NHT):
                        nc.tensor.matmul(fp_[:], w2t[:, a2_, :], hT[:, a2_, :],
                                         start=(a2_ == 0), stop=(a2_ == NHT - 1))
                    ot = fsm.tile([128, 512], F32, tag="ot")
                    nc.vector.scalar_tensor_tensor(ot[:], fp_[:], b2_sb[:, dt:dt + 1],
                                                   x1n[:, dt, :], ALU.add, ALU.add)
                    nc.sync.dma_start(
                        out_d[:, :].rearrange("(a p) n -> p a n", p=128)[:, dt, :], ot[:])
    nc.compile()
    return nc


_NC_CACHE = None


def _get_nc():
    global _NC_CACHE
    if _NC_CACHE is None:
        _NC_CACHE = _build_nc()
    return _NC_CACHE


def make_in_maps(inputs):
    x = np.asarray(inputs["x"], np.float32)
    Wq = np.asarray(inputs["Wq"], np.float32)
    Wk = np.asarray(inputs["Wk"], np.float32)
    Wv = np.asarray(inputs["Wv"], np.float32)
    Wproj = np.ascontiguousarray(np.asarray(inputs["Wproj"], np.float32))
    bproj = np.asarray(inputs["bproj"], np.float32)
    W1 = np.ascontiguousarray(np.asarray(inputs["W1"], np.float32))
    b1 = np.asarray(inputs["b1"], np.float32)
    W2 = np.ascontiguousarray(np.asarray(inputs["W2"], np.float32))
    b2 = np.asarray(inputs["b2"], np.float32)
    g1 = np.asarray(inputs["ln1_g"], np.float32)
    bl1 = np.asarray(inputs["ln1_b"], np.float32)
    g2 = np.asarray(inputs["ln2_g"], np.float32)
    bl2 = np.asarray(inputs["ln2_b"], np.float32)

    s = np.float32(D ** -0.5)
    # xt tiled: [NBLK, 128, NDC, 512]: block tb = flat tokens 512tb..512(tb+1),
    # chunk a = features 128a..128(a+1)
    x_flat = x.reshape(BT, D)
    xt = np.ascontiguousarray(
        x_flat.reshape(NBLK, 512, NDC, 128).transpose(0, 3, 2, 1)
    ).astype(ml_dtypes.bfloat16)

    masks = np.zeros((4, 128, 512), np.float32)
    jj = np.arange(512)[None, :]
    ii = np.arange(128)[:, None]
    for m in range(4):
        masks[m] = (jj >= m * 128 + ii).astype(np.float32)
    masks = masks.astype(ml_dtypes.bfloat16)

    common = {
        "xt": xt,
        "wproj": np.ascontiguousarray(Wproj.reshape(NDC, 128, D).transpose(1, 0, 2)).astype(ml_dtypes.bfloat16),
        "w1": np.ascontiguousarray(W1.reshape(NDC, 128, NHT, 128).transpose(2, 1, 0, 3)).astype(ml_dtypes.bfloat16),
        "w2": np.ascontiguousarray(W2.reshape(NHT, 128, NDC, 128).transpose(2, 1, 0, 3)).astype(ml_dtypes.bfloat16),
        "bproj_pp": np.ascontiguousarray(bproj.reshape(8, 128).T),
        "b1_pp": np.ascontiguousarray(b1.reshape(32, 128).T),
        "b2_pp": np.ascontiguousarray(b2.reshape(8, 128).T),
        "gb1": np.ascontiguousarray(np.stack([g1, bl1])),
        "gb2": np.ascontiguousarray(np.stack([g2, bl2])),
        "masks": masks,
        "ident": np.eye(128, dtype=np.float32),
        "ones_d": np.full((128, 1), 1.0 / D, np.float32),
        "ones_db": np.full((128, 1), 1.0 / D, ml_dtypes.bfloat16),
        "ones_1": np.ones((1, 128), np.float32),
    }

    in_maps = []
    for c in range(NC_):
        h0 = NH_LOC * c
        Wq_cat = np.concatenate([Wq[h0 + i] for i in range(NH_LOC)], 1)  # [D,128]
        Wk_cat = np.concatenate([Wk[h0 + i] for i in range(NH_LOC)], 1)
        Wv_cat = np.concatenate([Wv[h0 + i] for i in range(NH_LOC)], 1)
        Wq_eff = g1[:, None] * Wq_cat * s
        Wk_eff = g1[:, None] * Wk_cat
        Wv_eff = g1[:, None] * Wv_cat
        wqkv = np.ascontiguousarray(np.concatenate([Wq_eff, Wk_eff, Wv_eff], 1)).astype(ml_dtypes.bfloat16)
        wsums = np.concatenate([Wq_eff.sum(0), Wk_eff.sum(0), Wv_eff.sum(0)])
        wbias = np.concatenate([bl1 @ (Wq_cat * s), bl1 @ Wk_cat, bl1 @ Wv_cat])
        m = dict(common)
        xl = np.concatenate([x[0, 256 * c:256 * (c + 1)],
                             x[1, 256 * c:256 * (c + 1)]], axis=0)  # [512, D]
        m["xt_loc"] = np.ascontiguousarray(xl.reshape(512, NDC, 128).transpose(2, 1, 0))
        m["wqkv"] = wqkv
        m["wsb2"] = np.ascontiguousarray(
            np.stack([wsums, wbias]).astype(np.float32))
        in_maps.append(m)
    return in_maps


def run(inputs, trace=False, trace_kwargs=None):
    nc = _get_nc()
    in_maps = make_in_maps(inputs)
    res = run_bass_kernel_spmd(nc, in_maps, core_ids=list(range(NC_)),
                               trace=trace, **(trace_kwargs or {}))
    out = np.empty((B, T, D), np.float32)
    for c in range(NC_):
        o = res.results[c]["outT"]
        out[0, 256 * c:256 * (c + 1)] = o[:, 0:256].T
        out[1, 256 * c:256 * (c + 1)] = o[:, 256:512].T
    return out, res


def kernel(**inputs) -> np.ndarray:
    out, _ = run(inputs, trace=False)
    return out
